# revision 53
# baseline (speedup 1.0000x reference)
"""Trainium2 Bass kernel for nn_LocalReverseDiffusion (v5).

Reference computation (per sample n):
  y[n,c,d*4+i,h*4+j,w*4+k] = x[n,c,d,h,w] * w_ct[c,i,j,k] + b_ct[c]
  yn = GroupNorm(1 group, affine gamma/beta) over (C,D,H,W) of y
  out[n,o,:,:,:] = sum_c w_pw[o,c] * yn[n,c,:,:,:]

Identity: fold the whole chain into 64 small GEMMs (one per conv-transpose
offset (i,j,k)) applied to x directly:

  out[n,o,4d+i,4h+j,4w+k] = inv[n] * sum_c M0[ijk][o,c] * x[n,c,d,h,w] + C2[n,o]

  M0[ijk][o,c] = w_pw[o,c] * gamma[c] * w_ct[c,i,j,k]
  inv[n]       = rsqrt(var[n] + eps)
  C2[n,o]      = inv[n]*(K1[o] - mean[n]*K2[o]) + K3[o]      (K1/K2/K3 host consts)

GroupNorm stats closed form in per-(n,c) mean/E[x^2] of x (conv-transpose is
a non-overlapping scatter): bn_stats chunks over full x -> bn_aggr -> fold
over channels with tiny bf16 matmuls against host mask blocks (blocks carry
the 1/MT so the fold emits mean_tot / (E[y^2]+eps) directly).

Sharding: 8 cores, core cid owns input depth planes {2cid, 2cid+1} ->
output slab out[:, :, 8cid:8cid+8, :, :]. Output is written BF16 (the
harness tolerance is 2e-2; bf16 adds ~4e-3) and upcast to f32 on host ->
halves the 47us fp32 output-DMA floor to ~23.5us.

v5 perf structure (vs v4: 67us -> target ~36us):
 - Stats on ONE engine (DVE bn_stats x4 chunks + bn_aggr): frees ACT for
   its table load (sqrt set, loaded during the x DMA) and keeps the
   critical path load->stats->inv tight.
 - PSUM tiles per (j,dl): [128,1024] f32 = 2 banks, 4-slot ring (+ps_st in
   slot 0) = all 8 banks. Each PSUM tile is read by exactly ONE engine
   (scalar j0/j1, vector j2/j3) - cross-engine reads of a shared tile
   serialize in the Tile scheduler.
 - Copy order builds granule dl0 with BOTH engines first, then dl1, so the
   first 1MB output DMA issues ~2.7us after inv is ready and the DMA queue
   then stays fed (copies produce ~375 GB/s > 358 GB/s HBM).
 - PE order j0dl0,j2dl0,j1dl0,j3dl0,(dl1 same) matches copy drain order;
   PE prefills 3 tiles during stats so copies never wait on matmuls.
 - All input DMA triggers on the Sync ring, x chunks first (stats-critical),
   then xs/lt/swall/kk; output granule DMAs also Sync.
"""

import numpy as np
import ml_dtypes

import concourse.bass as bass
import concourse.mybir as mybir
import concourse.tile as tile
from concourse import bacc
from concourse.bass_utils import run_bass_kernel_spmd

# Problem shape (hardcoded per harness contract)
N, C, D, H, W = 2, 64, 16, 16, 16
R = 4
NCORES = 8
DL = D // NCORES            # input d-planes per core = 2
DO_PER_CORE = DL * R        # output do-planes per core = 8
EPS = 1e-5
MT = float(C * D * H * W * R**3)   # elements per GroupNorm group = 16777216
PV = float(D * H * W * R**3)       # positions per channel = 262144
ROW = float(D * H * W)             # elements per (n,c) row of x = 4096

F32 = mybir.dt.float32
BF16 = mybir.dt.bfloat16
AF = mybir.ActivationFunctionType
ALU = mybir.AluOpType

_CACHE = {}


def _build_program():
    nc = bacc.Bacc(
        "TRN2",
        target_bir_lowering=False,
        debug=False,
        enable_asserts=True,
        num_devices=NCORES,
    )

    # ---- DRAM I/O ----
    # xs2 rows = n*64+c so sample n lives on partitions n*64..n*64+63
    xs_d = nc.dram_tensor("xs", [N * C, DL * H * W], BF16, kind="ExternalInput")
    # chunk-major, 2 chunks of 512KB -> 4KB-per-partition descriptors
    # (2KB descriptors cap the input phase at ~215 GB/s)
    xfa_d = nc.dram_tensor("xfa", [N * C, 2048], BF16, kind="ExternalInput")
    xfb_d = nc.dram_tensor("xfb", [N * C, 1024], BF16, kind="ExternalInput")
    xfc_d = nc.dram_tensor("xfc", [N * C, 1024], BF16, kind="ExternalInput")
    # lt il-chunked so il0's weights load early while il1's wait until the
    # stats phase is done with the bandwidth; rows 64-127 (the n1 matmul
    # copies) are filled by SBUF->SBUF DMA instead of re-reading HBM
    lt_d = nc.dram_tensor("lt", [2, C, 2048], BF16, kind="ExternalInput")
    sw_d = nc.dram_tensor("swall", [128, 1280], BF16, kind="ExternalInput")
    kk_d = nc.dram_tensor("k123", [128, 3], F32, kind="ExternalInput")
    out_d = nc.dram_tensor(
        "out", [N, C, DO_PER_CORE, H * R, W * R], BF16, kind="ExternalOutput"
    )

    with tile.TileContext(nc) as tc:
        with (
            tc.tile_pool(name="consts", bufs=1) as consts,
            tc.tile_pool(name="xfp", bufs=1) as xfp,
            tc.tile_pool(name="stats", bufs=1) as stats,
            tc.tile_pool(name="ota", bufs=8) as otpa,
            tc.tile_pool(name="psum", bufs=4, space="PSUM") as psp,
        ):
            # ---- Input loads: sync ring carries xf (stats-critical, first)
            # then lt twice (same weights onto partitions 0-63 for n0 and
            # 64-127 for n1 -> PE row-group pairing) and kk; scalar ring
            # carries the small xs/swall behind its trigger slots.
            xf_t = xfp.tile([128, 4096], BF16)       # x as [(n c), dhw]
            nc.sync.dma_start(xf_t[:, 0:2048], xfa_d.ap())
            nc.sync.dma_start(xf_t[:, 2048:3072], xfb_d.ap())
            nc.sync.dma_start(xf_t[:, 3072:4096], xfc_d.ap())
            sw_t = consts.tile([128, 1280], BF16)
            nc.sync.dma_start(sw_t[:], sw_d.ap())
            kk_t = consts.tile([128, 3], F32)
            nc.sync.dma_start(kk_t[:], kk_d.ap())
            xs_t = consts.tile([N * C, DL * H * W], BF16)  # [(n c), (dl h w)]
            nc.scalar.dma_start(xs_t[:], xs_d.ap())
            lt_t = consts.tile([128, 4096], BF16)

            # ---- ACT table warm-up: sqrt anchors the set; square/identity
            # are 1-ULP fillers present in the same set. Loads during the x
            # DMA so the engine is table-ready well before inv/copies.
            warm = stats.tile([128, 2], F32)
            nc.vector.memset(warm[:], 1.0)
            nc.scalar.sqrt(warm[:, 0:1], warm[:, 0:1])
            nc.scalar.square(warm[:, 0:1], warm[:, 0:1])
            nc.scalar.activation(warm[:, 0:1], warm[:, 0:1], AF.Identity,
                                 bias=warm[:, 1:2], scale=warm[:, 1:2])
            # il0 weights behind the warmups: the trigger slot lands after
            # the x chunks have claimed the early HBM window; rows 64-127
            # duplicated on-chip (SBUF->SBUF, no HBM traffic)
            nc.scalar.dma_start(lt_t[0:64, 0:2048], lt_d.ap()[0])
            nc.scalar.dma_start(lt_t[64:128, 0:2048], lt_t[0:64, 0:2048])

            # ---- Stats: per-(n,c) mean/E[x^2] of x. DVE bn_stats (one pass
            # gives mean+var, 512-col max) covers cols 0:3072; ACT covers
            # the last 1024 cols (Identity-sum + Square-sumsq, scales baked
            # to 1/1024 so all P16 cols are "mean-like").
            bn6 = stats.tile([128, 6, 6], F32)
            for s in range(6):
                nc.vector.bn_stats(bn6[:, s], xf_t[:, s * 512 : (s + 1) * 512])
            mv = stats.tile([128, 2], F32)     # [mean_a, var_a] over 0:3072
            nc.vector.bn_aggr(mv[:], bn6[:])
            scr = xfp.tile([128, 1024], BF16)  # ACT throwaway out
            p8 = stats.tile([128, 2], F32)     # [s3, q3] = ch3 sum/sumsq /1024
            nc.scalar.activation(scr[:], xf_t[:, 3072:4096], AF.Identity,
                                 scale=1.0 / 1024.0, accum_out=p8[:, 0:1])
            nc.scalar.activation(scr[:], xf_t[:, 3072:4096], AF.Square,
                                 scale=1.0 / 32.0, accum_out=p8[:, 1:2])
            # combine bn side and ACT side into one "x1024" column each:
            # Sx/1024 = 3*mean_a + s3 ; Sxx/1024 = 3*(var_a+mean_a^2) + q3
            msq = stats.tile([128, 1], F32)
            nc.vector.tensor_mul(msq[:], mv[:, 0:1], mv[:, 0:1])
            ex2 = stats.tile([128, 1], F32)
            nc.vector.tensor_add(ex2[:], mv[:, 1:2], msq[:])
            P16 = stats.tile([128, 3], BF16)   # [Sx/1024, Sxx/1024, 1]
            nc.vector.memset(P16[:, 2:3], 1.0)
            nc.vector.tensor_scalar(P16[:, 0:1], mv[:, 0:1], 3.0, p8[:, 0:1],
                                    op0=ALU.mult, op1=ALU.add)
            nc.vector.tensor_scalar(P16[:, 1:2], ex2[:], 3.0, p8[:, 1:2],
                                    op0=ALU.mult, op1=ALU.add)

            # ---- Fold target (slot 0 of the PSUM ring). The fold matmuls
            # are EMITTED after the first main tile groups so the PE queue
            # doesn't stall on stats before doing any main work.
            # swall: 5 blocks per n ([128,128] each, all columns identical):
            #   b0: 1024*sw/MT*n0, b1: 1024*sww/MT*n0, b2: 1024*2b*sw/MT*n0,
            #   b3: (PV*b)/MT*n0, b4: (PV*b^2+eps*MT/C)/MT*n0; b5..b9 = n1.
            # ps_st cols: 0,1 = mean_tot(n0,n1); 2,3 = E[y^2]+eps (n0,n1).
            ps_st = psp.tile([128, 4], F32, tag="mm")

            def blk(i):
                return sw_t[:, i * 128 : (i + 1) * 128]

            def emit_folds():
                for nq in range(2):
                    o = 5 * nq
                    mc, ec = nq, 2 + nq
                    nc.tensor.matmul(ps_st[:, mc:mc+1], blk(o + 0), P16[:, 0:1],
                                     start=True, stop=False)
                    nc.tensor.matmul(ps_st[:, mc:mc+1], blk(o + 3), P16[:, 2:3],
                                     start=False, stop=True)
                    nc.tensor.matmul(ps_st[:, ec:ec+1], blk(o + 1), P16[:, 1:2],
                                     start=True, stop=False)
                    nc.tensor.matmul(ps_st[:, ec:ec+1], blk(o + 2), P16[:, 0:1],
                                     start=False, stop=False)
                    nc.tensor.matmul(ps_st[:, ec:ec+1], blk(o + 4), P16[:, 2:3],
                                     start=False, stop=True)

            def emit_statmath():
                # var = E[y^2]+eps - mean^2 ; inv = sqrt(1/var)
                # t1 = mean*K2 - K1 (ACT, per-partition kk); c2 = K3 - t1*inv
                msqt = stats.tile([128, 2], F32)
                nc.scalar.square(msqt[:], ps_st[:, 0:2])
                t1 = stats.tile([128, 2], F32)
                nc.scalar.activation(t1[:], ps_st[:, 0:2], AF.Identity,
                                     bias=kk_t[:, 0:1], scale=kk_t[:, 1:2])
                var_t = stats.tile([128, 2], F32)
                nc.vector.tensor_sub(var_t[:], ps_st[:, 2:4], msqt[:])
                rec_t = stats.tile([128, 2], F32)
                nc.vector.reciprocal(rec_t[:], var_t[:])
                inv_t = stats.tile([128, 2], F32)
                nc.scalar.sqrt(inv_t[:], rec_t[:])   # inv = sqrt(1/(var+eps))
                t2 = stats.tile([128, 2], F32)
                nc.vector.tensor_mul(t2[:], t1[:], inv_t[:])
                c2_t = stats.tile([128, 2], F32)
                nc.scalar.activation(c2_t[:], t2[:], AF.Identity,
                                     bias=kk_t[:, 2:3], scale=-1.0)  # K3 - t2
                # il1 weights: trigger after the stat chain so it doesn't
                # block the ACT ops feeding c2; il1 matmuls start much later
                nc.scalar.dma_start(lt_t[0:64, 2048:4096], lt_d.ap()[1])
                nc.scalar.dma_start(lt_t[64:128, 2048:4096],
                                    lt_t[0:64, 2048:4096])
                return inv_t, c2_t

            # ---- Main: 128 GEMMs (free=256) + affine copies + out DMA ----
            # lhsT layout: lt[:, pair*128 + 2*o + g] = M0[i=2*g+il, j, k][o, c]
            #   with pair = il*16 + j*4 + k,  psum partition p = 2*o + g.
            # PSUM tile per (n,il,j,dl): cols (k hs w); copy per (j,dl) reads
            # strided (hs, w, k) and writes the ot granule j-slice in 64-elem
            # stride-1 runs. ot granule (n,il,dl) cols: hs*256 + j*64 + w*4+k.
            out_ap = out_d.ap().rearrange(
                "n o (dl g il) ho wo -> n dl il o g (ho wo)", dl=DL, g=2, il=2
            )
            # n-PAIRED matmuls: sample n's weights/rhs live on partitions
            # n*64..n*64+63, so the (n0) and (n1) matmuls of the same
            # (il,j,k,dl) occupy PE row-groups 0 and 64 and run CONCURRENT
            # (tile_position auto-derived from base_partition) -> ~2 cols/cyc.
            # Tiles per (n,il,j,dl) = [128,1024] f32 = 2 banks; 4-slot ring.
            # Copies: scalar owns j0/j1 tiles, vector j2/j3 (one engine per
            # PSUM tile); drain order builds granule (n0,dl0) first.
            PAIRS = [(0, 0), (2, 0), (1, 0), (3, 0),
                     (0, 1), (2, 1), (1, 1), (3, 1)]
            inv_t = c2_t = None

            def emit_mm(ps, n, il, j, dl, k):
                pair = il * 16 + j * 4 + k
                nc.tensor.matmul(
                    ps[:, k * 256 : (k + 1) * 256],
                    lt_t[n * 64 : (n + 1) * 64,
                         pair * 128 : (pair + 1) * 128],
                    xs_t[n * 64 : (n + 1) * 64, dl * 256 : (dl + 1) * 256],
                    start=True, stop=True,
                )

            def emit_mms(ps, n, il, j, dl):
                for k in range(R):
                    emit_mm(ps, n, il, j, dl, k)

            def emit_pair(ps0, ps1, il, j, dl):
                # k-interleaved so the n0 (rows 0-63) and n1 (rows 64-127)
                # matmuls sit adjacent in the PE queue and run concurrently
                for k in range(R):
                    emit_mm(ps0, 0, il, j, dl, k)
                    emit_mm(ps1, 1, il, j, dl, k)

            for il in range(2):
                ots = {}
                for n in range(N):
                    for dl in range(DL):
                        ots[(n, dl)] = otpa.tile(
                            [128, 4096], BF16, tag="ota",
                            name=f"ot_{n}_{il}_{dl}")
                ps_map = {}
                for pi, (j, dl) in enumerate(PAIRS):
                    for n in range(N):
                        ps = psp.tile([128, 1024], F32, tag="mm",
                                      name=f"ps_{il}_{j}_{dl}_{n}")
                        ps_map[(j, dl, n)] = ps
                    if il == 0 and pi == 1:
                        # folds right after the first 4 main matmuls so the
                        # stat chain isn't queued behind the whole prefill;
                        # the tile landing in ps_st's ring slot then has its
                        # WAR-release already in the queue (else deadlock).
                        j0, dl0 = PAIRS[0]
                        emit_mms(ps_map[(j0, dl0, 0)], 0, il, j0, dl0)
                        emit_folds()
                        inv_t, c2_t = emit_statmath()
                        emit_mms(ps_map[(j0, dl0, 1)], 1, il, j0, dl0)
                        emit_pair(ps_map[(j, dl, 0)], ps_map[(j, dl, 1)],
                                  il, j, dl)
                    elif il == 0 and pi == 0:
                        pass  # deferred: emitted right before the folds
                    else:
                        emit_pair(ps_map[(j, dl, 0)], ps_map[(j, dl, 1)],
                                  il, j, dl)
                # copies + granule DMAs; granule (n, dl) needs scalar copies
                # of (j0/j1, dl, n) and vector copies of (j2/j3, dl, n).
                # n-interleaved per j so each tile's WAR-release matches the
                # ring order (no PE queue stalls on later pairs).
                for dl in range(DL):
                    for j in range(R):
                        for n in range(N):
                            src = ps_map[(j, dl, n)][:].rearrange(
                                "p (k hs w) -> p hs w k", k=R, hs=H, w=W
                            )
                            dst = ots[(n, dl)][:].rearrange(
                                "p (hs j w k) -> p hs j w k",
                                hs=H, j=R, w=W, k=R
                            )[:, :, j]
                            if j < 2:
                                nc.scalar.activation(
                                    dst, src, AF.Identity,
                                    bias=c2_t[:, n : n + 1],
                                    scale=inv_t[:, n : n + 1],
                                )
                            else:
                                nc.vector.tensor_scalar(
                                    dst, src,
                                    inv_t[:, n : n + 1], c2_t[:, n : n + 1],
                                    op0=ALU.mult, op1=ALU.add,
                                )
                    for n in range(N):
                        nc.sync.dma_start(out_ap[n, dl, il], ots[(n, dl)][:])

    nc.compile()
    return nc


def _host_consts(w_ct, b_ct, gamma, beta, w_pw):
    w_ct = np.asarray(w_ct, np.float32).reshape(C, R, R, R)
    b_ct = np.asarray(b_ct, np.float32)
    gamma = np.asarray(gamma, np.float32)
    beta = np.asarray(beta, np.float32)
    w_pw = np.asarray(w_pw, np.float32).reshape(C, C)  # [o, c]

    gw = gamma[:, None, None, None] * w_ct  # [c, i, j, k]
    # lt [c, il, j, k, o, g]; i = 2*g + il; col = pair*128 + 2*o + g
    sc_g0 = gw[:, 0:2]  # g=0: i = il in {0, 1}
    sc_g1 = gw[:, 2:4]  # g=1: i = 2+il
    sc = np.stack([sc_g0, sc_g1], axis=4)  # [c, il, j, k, g]
    lt = (sc[:, :, :, :, None, :]
          * w_pw.T[:, None, None, None, :, None]).reshape(C, 4096)
    # il-chunk-major for the staged loads (row duplication happens on-chip)
    lt = np.ascontiguousarray(
        lt.reshape(C, 2, 2048).transpose(1, 0, 2)
    ).astype(ml_dtypes.bfloat16)

    wflat = w_ct.reshape(C, -1)
    sw = 1024.0 * wflat.sum(1) / MT
    sww = 1024.0 * (wflat**2).sum(1) / MT
    tbsw = 1024.0 * 2.0 * b_ct * wflat.sum(1) / MT
    cb = PV * b_ct / MT
    # EPS spread over the C ones so the fold emits E[y^2]+eps directly
    cb2 = (PV * b_ct**2) / MT + EPS / C
    blocks = []
    for nq in range(2):
        for vec in (sw, sww, tbsw, cb, cb2):
            v = np.zeros(128, np.float32)
            v[nq * 64 : (nq + 1) * 64] = vec
            blocks.append(np.repeat(v[:, None], 128, axis=1))
    swall = np.concatenate(blocks, axis=1).astype(ml_dtypes.bfloat16)

    # K1[o]=sum_c wpw*gamma*b, K2[o]=sum_c wpw*gamma, K3[o]=sum_c wpw*beta,
    # expanded to partitions p = 2*o + g. kk col0 = -K1 (ACT bias for
    # t1 = mean*K2 - K1), col1 = K2, col2 = K3.
    k1 = w_pw @ (gamma * b_ct)
    k2 = w_pw @ gamma
    k3 = w_pw @ beta
    k123 = np.repeat(np.stack([-k1, k2, k3], axis=1), 2, axis=0)
    k123 = np.ascontiguousarray(k123, np.float32)
    return lt, swall, k123


def _get_nc():
    if "nc" not in _CACHE:
        _CACHE["nc"] = _build_program()
    return _CACHE["nc"]


def make_in_maps(x, w_ct, b_ct, gamma, beta, w_pw):
    x = np.ascontiguousarray(np.asarray(x, np.float32))
    lt, swall, k123 = _host_consts(w_ct, b_ct, gamma, beta, w_pw)
    x16 = x.astype(ml_dtypes.bfloat16)
    xr = x16.reshape(N * C, 4096)
    xfa = np.ascontiguousarray(xr[:, 0:2048])
    xfb = np.ascontiguousarray(xr[:, 2048:3072])
    xfc = np.ascontiguousarray(xr[:, 3072:4096])
    in_maps = []
    for cid in range(NCORES):
        # xs [(n c), (dl h w)]: core's 2 depth planes, sample-major rows so
        # sample n sits on partitions n*64..n*64+63 (PE row-group pairing)
        xs = np.ascontiguousarray(
            x16[:, :, 2 * cid : 2 * cid + 2].reshape(N * C, DL * H * W)
        )
        in_maps.append(dict(xs=xs, xfa=xfa, xfb=xfb, xfc=xfc,
                            lt=lt, swall=swall, k123=k123))
    return in_maps


def assemble(results):
    return np.concatenate(
        [results[cid]["out"] for cid in range(NCORES)], axis=2
    ).astype(np.float32)


def kernel(x, w_ct, b_ct, gamma, beta, w_pw):
    nc = _get_nc()
    in_maps = make_in_maps(x, w_ct, b_ct, gamma, beta, w_pw)
    res = run_bass_kernel_spmd(nc, in_maps, list(range(NCORES))).results
    return assemble(res)


# revision 61
# speedup vs baseline: 1.0116x; 1.0116x over previous
"""Trainium2 Bass kernel for nn_LocalReverseDiffusion (v5).

Reference computation (per sample n):
  y[n,c,d*4+i,h*4+j,w*4+k] = x[n,c,d,h,w] * w_ct[c,i,j,k] + b_ct[c]
  yn = GroupNorm(1 group, affine gamma/beta) over (C,D,H,W) of y
  out[n,o,:,:,:] = sum_c w_pw[o,c] * yn[n,c,:,:,:]

Identity: fold the whole chain into 64 small GEMMs (one per conv-transpose
offset (i,j,k)) applied to x directly:

  out[n,o,4d+i,4h+j,4w+k] = inv[n] * sum_c M0[ijk][o,c] * x[n,c,d,h,w] + C2[n,o]

  M0[ijk][o,c] = w_pw[o,c] * gamma[c] * w_ct[c,i,j,k]
  inv[n]       = rsqrt(var[n] + eps)
  C2[n,o]      = inv[n]*(K1[o] - mean[n]*K2[o]) + K3[o]      (K1/K2/K3 host consts)

GroupNorm stats closed form in per-(n,c) mean/E[x^2] of x (conv-transpose is
a non-overlapping scatter): bn_stats chunks over full x -> bn_aggr -> fold
over channels with tiny bf16 matmuls against host mask blocks (blocks carry
the 1/MT so the fold emits mean_tot / (E[y^2]+eps) directly).

Sharding: 8 cores, core cid owns input depth planes {2cid, 2cid+1} ->
output slab out[:, :, 8cid:8cid+8, :, :]. Output is written BF16 (the
harness tolerance is 2e-2; bf16 adds ~4e-3) and upcast to f32 on host ->
halves the 47us fp32 output-DMA floor to ~23.5us.

v5 perf structure (vs v4: 67us -> target ~36us):
 - Stats on ONE engine (DVE bn_stats x4 chunks + bn_aggr): frees ACT for
   its table load (sqrt set, loaded during the x DMA) and keeps the
   critical path load->stats->inv tight.
 - PSUM tiles per (j,dl): [128,1024] f32 = 2 banks, 4-slot ring (+ps_st in
   slot 0) = all 8 banks. Each PSUM tile is read by exactly ONE engine
   (scalar j0/j1, vector j2/j3) - cross-engine reads of a shared tile
   serialize in the Tile scheduler.
 - Copy order builds granule dl0 with BOTH engines first, then dl1, so the
   first 1MB output DMA issues ~2.7us after inv is ready and the DMA queue
   then stays fed (copies produce ~375 GB/s > 358 GB/s HBM).
 - PE order j0dl0,j2dl0,j1dl0,j3dl0,(dl1 same) matches copy drain order;
   PE prefills 3 tiles during stats so copies never wait on matmuls.
 - All input DMA triggers on the Sync ring, x chunks first (stats-critical),
   then xs/lt/swall/kk; output granule DMAs also Sync.
"""

import numpy as np
import ml_dtypes

import concourse.bass as bass
import concourse.mybir as mybir
import concourse.tile as tile
from concourse import bacc
from concourse.bass_utils import run_bass_kernel_spmd

# Problem shape (hardcoded per harness contract)
N, C, D, H, W = 2, 64, 16, 16, 16
R = 4
NCORES = 8
DL = D // NCORES            # input d-planes per core = 2
DO_PER_CORE = DL * R        # output do-planes per core = 8
EPS = 1e-5
MT = float(C * D * H * W * R**3)   # elements per GroupNorm group = 16777216
PV = float(D * H * W * R**3)       # positions per channel = 262144
ROW = float(D * H * W)             # elements per (n,c) row of x = 4096

F32 = mybir.dt.float32
BF16 = mybir.dt.bfloat16
AF = mybir.ActivationFunctionType
ALU = mybir.AluOpType

_CACHE = {}


def _build_program():
    nc = bacc.Bacc(
        "TRN2",
        target_bir_lowering=False,
        debug=False,
        enable_asserts=True,
        num_devices=NCORES,
    )

    # ---- DRAM I/O ----
    # xs2 rows = n*64+c so sample n lives on partitions n*64..n*64+63
    xs_d = nc.dram_tensor("xs", [N * C, DL * H * W], BF16, kind="ExternalInput")
    # chunk-major, 2 chunks of 512KB -> 4KB-per-partition descriptors
    # (2KB descriptors cap the input phase at ~215 GB/s)
    xf_d = nc.dram_tensor("xf", [2, N * C, 2048], BF16, kind="ExternalInput")
    # lt host-duplicated to 128 rows (64-partition DMAs engage half the
    # SDMA engines) and il-chunked so il0's weights can load early while
    # il1's wait until the stats phase is done with the bandwidth
    lt_d = nc.dram_tensor("lt", [2, 2 * C, 2048], BF16, kind="ExternalInput")
    sw_d = nc.dram_tensor("swall", [128, 1280], BF16, kind="ExternalInput")
    kk_d = nc.dram_tensor("k123", [128, 3], F32, kind="ExternalInput")
    out_d = nc.dram_tensor(
        "out", [N, C, DO_PER_CORE, H * R, W * R], BF16, kind="ExternalOutput"
    )

    with tile.TileContext(nc) as tc:
        with (
            tc.tile_pool(name="consts", bufs=1) as consts,
            tc.tile_pool(name="xfp", bufs=1) as xfp,
            tc.tile_pool(name="stats", bufs=1) as stats,
            tc.tile_pool(name="ota", bufs=8) as otpa,
            tc.tile_pool(name="psum", bufs=4, space="PSUM") as psp,
        ):
            # ---- Input loads: sync ring carries xf (stats-critical, first)
            # then lt twice (same weights onto partitions 0-63 for n0 and
            # 64-127 for n1 -> PE row-group pairing) and kk; scalar ring
            # carries the small xs/swall behind its trigger slots.
            xf_t = xfp.tile([128, 4096], BF16)       # x as [(n c), dhw]
            for ch in range(2):
                nc.sync.dma_start(xf_t[:, ch * 2048 : (ch + 1) * 2048],
                                  xf_d.ap()[ch])
            sw_t = consts.tile([128, 1280], BF16)
            nc.sync.dma_start(sw_t[:], sw_d.ap())
            kk_t = consts.tile([128, 3], F32)
            nc.sync.dma_start(kk_t[:], kk_d.ap())
            xs_t = consts.tile([N * C, DL * H * W], BF16)  # [(n c), (dl h w)]
            nc.scalar.dma_start(xs_t[:], xs_d.ap())
            lt_t = consts.tile([128, 4096], BF16)

            # ---- ACT table warm-up: sqrt anchors the set; square/identity
            # are 1-ULP fillers present in the same set. Loads during the x
            # DMA so the engine is table-ready well before inv/copies.
            warm = stats.tile([128, 2], F32)
            nc.vector.memset(warm[:], 1.0)
            nc.scalar.sqrt(warm[:, 0:1], warm[:, 0:1])
            nc.scalar.square(warm[:, 0:1], warm[:, 0:1])
            nc.scalar.activation(warm[:, 0:1], warm[:, 0:1], AF.Identity,
                                 bias=warm[:, 1:2], scale=warm[:, 1:2])
            # il0 weights behind the warmups: the trigger slot lands after
            # the x chunks have claimed the early HBM window
            nc.scalar.dma_start(lt_t[:, 0:2048], lt_d.ap()[0])

            # ---- Stats: per-(n,c) mean/E[x^2] of x. DVE bn_stats (one pass
            # gives mean+var, 512-col max) covers cols 0:3072; ACT covers
            # the last 1024 cols (Identity-sum + Square-sumsq, scales baked
            # to 1/1024 so all P16 cols are "mean-like").
            bn6 = stats.tile([128, 6, 6], F32)
            for s in range(6):
                nc.vector.bn_stats(bn6[:, s], xf_t[:, s * 512 : (s + 1) * 512])
            mv = stats.tile([128, 2], F32)     # [mean_a, var_a] over 0:3072
            nc.vector.bn_aggr(mv[:], bn6[:])
            scr = xfp.tile([128, 1024], BF16)  # ACT throwaway out
            p8 = stats.tile([128, 2], F32)     # [s3, q3] = ch3 sum/sumsq /1024
            nc.scalar.activation(scr[:], xf_t[:, 3072:4096], AF.Identity,
                                 scale=1.0 / 1024.0, accum_out=p8[:, 0:1])
            nc.scalar.activation(scr[:], xf_t[:, 3072:4096], AF.Square,
                                 scale=1.0 / 32.0, accum_out=p8[:, 1:2])
            # combine bn side and ACT side into one "x1024" column each:
            # Sx/1024 = 3*mean_a + s3 ; Sxx/1024 = 3*(var_a+mean_a^2) + q3
            msq = stats.tile([128, 1], F32)
            nc.vector.tensor_mul(msq[:], mv[:, 0:1], mv[:, 0:1])
            ex2 = stats.tile([128, 1], F32)
            nc.vector.tensor_add(ex2[:], mv[:, 1:2], msq[:])
            P16 = stats.tile([128, 3], BF16)   # [Sx/1024, Sxx/1024, 1]
            nc.vector.memset(P16[:, 2:3], 1.0)
            nc.vector.tensor_scalar(P16[:, 0:1], mv[:, 0:1], 3.0, p8[:, 0:1],
                                    op0=ALU.mult, op1=ALU.add)
            nc.vector.tensor_scalar(P16[:, 1:2], ex2[:], 3.0, p8[:, 1:2],
                                    op0=ALU.mult, op1=ALU.add)

            # ---- Fold target (slot 0 of the PSUM ring). The fold matmuls
            # are EMITTED after the first main tile groups so the PE queue
            # doesn't stall on stats before doing any main work.
            # swall: 5 blocks per n ([128,128] each, all columns identical):
            #   b0: 1024*sw/MT*n0, b1: 1024*sww/MT*n0, b2: 1024*2b*sw/MT*n0,
            #   b3: (PV*b)/MT*n0, b4: (PV*b^2+eps*MT/C)/MT*n0; b5..b9 = n1.
            # ps_st cols: 0,1 = mean_tot(n0,n1); 2,3 = E[y^2]+eps (n0,n1).
            ps_st = psp.tile([128, 4], F32, tag="mm")

            def blk(i):
                return sw_t[:, i * 128 : (i + 1) * 128]

            def emit_folds():
                for nq in range(2):
                    o = 5 * nq
                    mc, ec = nq, 2 + nq
                    nc.tensor.matmul(ps_st[:, mc:mc+1], blk(o + 0), P16[:, 0:1],
                                     start=True, stop=False)
                    nc.tensor.matmul(ps_st[:, mc:mc+1], blk(o + 3), P16[:, 2:3],
                                     start=False, stop=True)
                    nc.tensor.matmul(ps_st[:, ec:ec+1], blk(o + 1), P16[:, 1:2],
                                     start=True, stop=False)
                    nc.tensor.matmul(ps_st[:, ec:ec+1], blk(o + 2), P16[:, 0:1],
                                     start=False, stop=False)
                    nc.tensor.matmul(ps_st[:, ec:ec+1], blk(o + 4), P16[:, 2:3],
                                     start=False, stop=True)

            def emit_statmath():
                # var = E[y^2]+eps - mean^2 ; inv = sqrt(1/var)
                # t1 = mean*K2 - K1 (ACT, per-partition kk); c2 = K3 - t1*inv
                msqt = stats.tile([128, 2], F32)
                nc.scalar.square(msqt[:], ps_st[:, 0:2])
                t1 = stats.tile([128, 2], F32)
                nc.scalar.activation(t1[:], ps_st[:, 0:2], AF.Identity,
                                     bias=kk_t[:, 0:1], scale=kk_t[:, 1:2])
                var_t = stats.tile([128, 2], F32)
                nc.vector.tensor_sub(var_t[:], ps_st[:, 2:4], msqt[:])
                rec_t = stats.tile([128, 2], F32)
                nc.vector.reciprocal(rec_t[:], var_t[:])
                inv_t = stats.tile([128, 2], F32)
                nc.scalar.sqrt(inv_t[:], rec_t[:])   # inv = sqrt(1/(var+eps))
                t2 = stats.tile([128, 2], F32)
                nc.vector.tensor_mul(t2[:], t1[:], inv_t[:])
                c2_t = stats.tile([128, 2], F32)
                nc.scalar.activation(c2_t[:], t2[:], AF.Identity,
                                     bias=kk_t[:, 2:3], scale=-1.0)  # K3 - t2
                # il1 weights: trigger after the stat chain so it doesn't
                # block the ACT ops feeding c2; il1 matmuls start much later
                nc.scalar.dma_start(lt_t[:, 2048:4096], lt_d.ap()[1])
                return inv_t, c2_t

            # ---- Main: 128 GEMMs (free=256) + affine copies + out DMA ----
            # lhsT layout: lt[:, pair*128 + 2*o + g] = M0[i=2*g+il, j, k][o, c]
            #   with pair = il*16 + j*4 + k,  psum partition p = 2*o + g.
            # PSUM tile per (n,il,j,dl): cols (k hs w); copy per (j,dl) reads
            # strided (hs, w, k) and writes the ot granule j-slice in 64-elem
            # stride-1 runs. ot granule (n,il,dl) cols: hs*256 + j*64 + w*4+k.
            out_ap = out_d.ap().rearrange(
                "n o (dl g il) ho wo -> n dl il o g (ho wo)", dl=DL, g=2, il=2
            )
            # n-PAIRED matmuls: sample n's weights/rhs live on partitions
            # n*64..n*64+63, so the (n0) and (n1) matmuls of the same
            # (il,j,k,dl) occupy PE row-groups 0 and 64 and run CONCURRENT
            # (tile_position auto-derived from base_partition) -> ~2 cols/cyc.
            # Tiles per (n,il,j,dl) = [128,1024] f32 = 2 banks; 4-slot ring.
            # Copies: scalar owns j0/j1 tiles, vector j2/j3 (one engine per
            # PSUM tile); drain order builds granule (n0,dl0) first.
            PAIRS = [(0, 0), (2, 0), (1, 0), (3, 0),
                     (0, 1), (2, 1), (1, 1), (3, 1)]
            inv_t = c2_t = None

            def emit_mm(ps, n, il, j, dl, k):
                pair = il * 16 + j * 4 + k
                nc.tensor.matmul(
                    ps[:, k * 256 : (k + 1) * 256],
                    lt_t[n * 64 : (n + 1) * 64,
                         pair * 128 : (pair + 1) * 128],
                    xs_t[n * 64 : (n + 1) * 64, dl * 256 : (dl + 1) * 256],
                    start=True, stop=True,
                )

            def emit_mms(ps, n, il, j, dl):
                for k in range(R):
                    emit_mm(ps, n, il, j, dl, k)

            def emit_pair(ps0, ps1, il, j, dl):
                # k-interleaved so the n0 (rows 0-63) and n1 (rows 64-127)
                # matmuls sit adjacent in the PE queue and run concurrently
                for k in range(R):
                    emit_mm(ps0, 0, il, j, dl, k)
                    emit_mm(ps1, 1, il, j, dl, k)

            for il in range(2):
                ots = {}
                for n in range(N):
                    for dl in range(DL):
                        ots[(n, dl)] = otpa.tile(
                            [128, 4096], BF16, tag="ota",
                            name=f"ot_{n}_{il}_{dl}")
                ps_map = {}
                for pi, (j, dl) in enumerate(PAIRS):
                    for n in range(N):
                        ps = psp.tile([128, 1024], F32, tag="mm",
                                      name=f"ps_{il}_{j}_{dl}_{n}")
                        ps_map[(j, dl, n)] = ps
                    if il == 0 and pi == 1:
                        # folds right after the first 4 main matmuls so the
                        # stat chain isn't queued behind the whole prefill;
                        # the tile landing in ps_st's ring slot then has its
                        # WAR-release already in the queue (else deadlock).
                        j0, dl0 = PAIRS[0]
                        emit_mms(ps_map[(j0, dl0, 0)], 0, il, j0, dl0)
                        emit_folds()
                        inv_t, c2_t = emit_statmath()
                        emit_mms(ps_map[(j0, dl0, 1)], 1, il, j0, dl0)
                        emit_pair(ps_map[(j, dl, 0)], ps_map[(j, dl, 1)],
                                  il, j, dl)
                    elif il == 0 and pi == 0:
                        pass  # deferred: emitted right before the folds
                    else:
                        emit_pair(ps_map[(j, dl, 0)], ps_map[(j, dl, 1)],
                                  il, j, dl)
                # copies + granule DMAs; granule (n, dl) needs scalar copies
                # of (j0/j1, dl, n) and vector copies of (j2/j3, dl, n).
                # n-interleaved per j so each tile's WAR-release matches the
                # ring order (no PE queue stalls on later pairs).
                for dl in range(DL):
                    for j in range(R):
                        for n in range(N):
                            src = ps_map[(j, dl, n)][:].rearrange(
                                "p (k hs w) -> p hs w k", k=R, hs=H, w=W
                            )
                            dst = ots[(n, dl)][:].rearrange(
                                "p (hs j w k) -> p hs j w k",
                                hs=H, j=R, w=W, k=R
                            )[:, :, j]
                            if j < 2:
                                nc.scalar.activation(
                                    dst, src, AF.Identity,
                                    bias=c2_t[:, n : n + 1],
                                    scale=inv_t[:, n : n + 1],
                                )
                            else:
                                nc.vector.tensor_scalar(
                                    dst, src,
                                    inv_t[:, n : n + 1], c2_t[:, n : n + 1],
                                    op0=ALU.mult, op1=ALU.add,
                                )
                    for n in range(N):
                        nc.sync.dma_start(out_ap[n, dl, il], ots[(n, dl)][:])

    nc.compile()
    return nc


def _host_consts(w_ct, b_ct, gamma, beta, w_pw):
    w_ct = np.asarray(w_ct, np.float32).reshape(C, R, R, R)
    b_ct = np.asarray(b_ct, np.float32)
    gamma = np.asarray(gamma, np.float32)
    beta = np.asarray(beta, np.float32)
    w_pw = np.asarray(w_pw, np.float32).reshape(C, C)  # [o, c]

    gw = gamma[:, None, None, None] * w_ct  # [c, i, j, k]
    # lt [c, il, j, k, o, g]; i = 2*g + il; col = pair*128 + 2*o + g
    sc_g0 = gw[:, 0:2]  # g=0: i = il in {0, 1}
    sc_g1 = gw[:, 2:4]  # g=1: i = 2+il
    sc = np.stack([sc_g0, sc_g1], axis=4)  # [c, il, j, k, g]
    lt = (sc[:, :, :, :, None, :]
          * w_pw.T[:, None, None, None, :, None]).reshape(C, 4096)
    # duplicated onto rows 64-127 for the n1 row-group matmuls, then
    # il-chunk-major for the staged loads
    lt = np.concatenate([lt, lt], axis=0)           # [128, 4096]
    lt = np.ascontiguousarray(
        lt.reshape(2 * C, 2, 2048).transpose(1, 0, 2)
    ).astype(ml_dtypes.bfloat16)

    wflat = w_ct.reshape(C, -1)
    sw = 1024.0 * wflat.sum(1) / MT
    sww = 1024.0 * (wflat**2).sum(1) / MT
    tbsw = 1024.0 * 2.0 * b_ct * wflat.sum(1) / MT
    cb = PV * b_ct / MT
    # EPS spread over the C ones so the fold emits E[y^2]+eps directly
    cb2 = (PV * b_ct**2) / MT + EPS / C
    blocks = []
    for nq in range(2):
        for vec in (sw, sww, tbsw, cb, cb2):
            v = np.zeros(128, np.float32)
            v[nq * 64 : (nq + 1) * 64] = vec
            blocks.append(np.repeat(v[:, None], 128, axis=1))
    swall = np.concatenate(blocks, axis=1).astype(ml_dtypes.bfloat16)

    # K1[o]=sum_c wpw*gamma*b, K2[o]=sum_c wpw*gamma, K3[o]=sum_c wpw*beta,
    # expanded to partitions p = 2*o + g. kk col0 = -K1 (ACT bias for
    # t1 = mean*K2 - K1), col1 = K2, col2 = K3.
    k1 = w_pw @ (gamma * b_ct)
    k2 = w_pw @ gamma
    k3 = w_pw @ beta
    k123 = np.repeat(np.stack([-k1, k2, k3], axis=1), 2, axis=0)
    k123 = np.ascontiguousarray(k123, np.float32)
    return lt, swall, k123


def _get_nc():
    if "nc" not in _CACHE:
        _CACHE["nc"] = _build_program()
    return _CACHE["nc"]


def make_in_maps(x, w_ct, b_ct, gamma, beta, w_pw):
    x = np.ascontiguousarray(np.asarray(x, np.float32))
    lt, swall, k123 = _host_consts(w_ct, b_ct, gamma, beta, w_pw)
    x16 = x.astype(ml_dtypes.bfloat16)
    xf = np.ascontiguousarray(
        x16.reshape(N * C, 2, 2048).transpose(1, 0, 2)
    )
    in_maps = []
    for cid in range(NCORES):
        # xs [(n c), (dl h w)]: core's 2 depth planes, sample-major rows so
        # sample n sits on partitions n*64..n*64+63 (PE row-group pairing)
        xs = np.ascontiguousarray(
            x16[:, :, 2 * cid : 2 * cid + 2].reshape(N * C, DL * H * W)
        )
        in_maps.append(dict(xs=xs, xf=xf, lt=lt, swall=swall, k123=k123))
    return in_maps


def assemble(results):
    return np.concatenate(
        [results[cid]["out"] for cid in range(NCORES)], axis=2
    ).astype(np.float32)


def kernel(x, w_ct, b_ct, gamma, beta, w_pw):
    nc = _get_nc()
    in_maps = make_in_maps(x, w_ct, b_ct, gamma, beta, w_pw)
    res = run_bass_kernel_spmd(nc, in_maps, list(range(NCORES))).results
    return assemble(res)


# revision 62
# speedup vs baseline: 1.0332x; 1.0214x over previous
"""Trainium2 Bass kernel for nn_LocalReverseDiffusion (v5).

Reference computation (per sample n):
  y[n,c,d*4+i,h*4+j,w*4+k] = x[n,c,d,h,w] * w_ct[c,i,j,k] + b_ct[c]
  yn = GroupNorm(1 group, affine gamma/beta) over (C,D,H,W) of y
  out[n,o,:,:,:] = sum_c w_pw[o,c] * yn[n,c,:,:,:]

Identity: fold the whole chain into 64 small GEMMs (one per conv-transpose
offset (i,j,k)) applied to x directly:

  out[n,o,4d+i,4h+j,4w+k] = inv[n] * sum_c M0[ijk][o,c] * x[n,c,d,h,w] + C2[n,o]

  M0[ijk][o,c] = w_pw[o,c] * gamma[c] * w_ct[c,i,j,k]
  inv[n]       = rsqrt(var[n] + eps)
  C2[n,o]      = inv[n]*(K1[o] - mean[n]*K2[o]) + K3[o]      (K1/K2/K3 host consts)

GroupNorm stats closed form in per-(n,c) mean/E[x^2] of x (conv-transpose is
a non-overlapping scatter): bn_stats chunks over full x -> bn_aggr -> fold
over channels with tiny bf16 matmuls against host mask blocks (blocks carry
the 1/MT so the fold emits mean_tot / (E[y^2]+eps) directly).

Sharding: 8 cores, core cid owns input depth planes {2cid, 2cid+1} ->
output slab out[:, :, 8cid:8cid+8, :, :]. Output is written BF16 (the
harness tolerance is 2e-2; bf16 adds ~4e-3) and upcast to f32 on host ->
halves the 47us fp32 output-DMA floor to ~23.5us.

v5 perf structure (vs v4: 67us -> target ~36us):
 - Stats on ONE engine (DVE bn_stats x4 chunks + bn_aggr): frees ACT for
   its table load (sqrt set, loaded during the x DMA) and keeps the
   critical path load->stats->inv tight.
 - PSUM tiles per (j,dl): [128,1024] f32 = 2 banks, 4-slot ring (+ps_st in
   slot 0) = all 8 banks. Each PSUM tile is read by exactly ONE engine
   (scalar j0/j1, vector j2/j3) - cross-engine reads of a shared tile
   serialize in the Tile scheduler.
 - Copy order builds granule dl0 with BOTH engines first, then dl1, so the
   first 1MB output DMA issues ~2.7us after inv is ready and the DMA queue
   then stays fed (copies produce ~375 GB/s > 358 GB/s HBM).
 - PE order j0dl0,j2dl0,j1dl0,j3dl0,(dl1 same) matches copy drain order;
   PE prefills 3 tiles during stats so copies never wait on matmuls.
 - All input DMA triggers on the Sync ring, x chunks first (stats-critical),
   then xs/lt/swall/kk; output granule DMAs also Sync.
"""

import numpy as np
import ml_dtypes

import concourse.bass as bass
import concourse.mybir as mybir
import concourse.tile as tile
from concourse import bacc
from concourse.bass_utils import run_bass_kernel_spmd

# Problem shape (hardcoded per harness contract)
N, C, D, H, W = 2, 64, 16, 16, 16
R = 4
NCORES = 8
DL = D // NCORES            # input d-planes per core = 2
DO_PER_CORE = DL * R        # output do-planes per core = 8
EPS = 1e-5
MT = float(C * D * H * W * R**3)   # elements per GroupNorm group = 16777216
PV = float(D * H * W * R**3)       # positions per channel = 262144
ROW = float(D * H * W)             # elements per (n,c) row of x = 4096

F32 = mybir.dt.float32
BF16 = mybir.dt.bfloat16
AF = mybir.ActivationFunctionType
ALU = mybir.AluOpType

_CACHE = {}


def _build_program():
    nc = bacc.Bacc(
        "TRN2",
        target_bir_lowering=False,
        debug=False,
        enable_asserts=True,
        num_devices=NCORES,
    )

    # ---- DRAM I/O ----
    # xs2 rows = n*64+c so sample n lives on partitions n*64..n*64+63
    xs_d = nc.dram_tensor("xs", [N * C, DL * H * W], BF16, kind="ExternalInput")
    # chunk-major, 2 chunks of 512KB -> 4KB-per-partition descriptors
    # (2KB descriptors cap the input phase at ~215 GB/s)
    xf_d = nc.dram_tensor("xf", [2, N * C, 2048], BF16, kind="ExternalInput")
    # lt host-duplicated to 128 rows (64-partition DMAs engage half the
    # SDMA engines) and il-chunked so il0's weights can load early while
    # il1's wait until the stats phase is done with the bandwidth
    lt_d = nc.dram_tensor("lt", [2, 2 * C, 2048], BF16, kind="ExternalInput")
    sw_d = nc.dram_tensor("swall", [128, 1280], BF16, kind="ExternalInput")
    kk_d = nc.dram_tensor("k123", [128, 3], F32, kind="ExternalInput")
    out_d = nc.dram_tensor(
        "out", [N, C, DO_PER_CORE, H * R, W * R], BF16, kind="ExternalOutput"
    )

    with tile.TileContext(nc) as tc:
        with (
            tc.tile_pool(name="consts", bufs=1) as consts,
            tc.tile_pool(name="xfp", bufs=1) as xfp,
            tc.tile_pool(name="stats", bufs=1) as stats,
            tc.tile_pool(name="ota", bufs=8) as otpa,
            tc.tile_pool(name="psum", bufs=4, space="PSUM") as psp,
        ):
            # ---- Input loads: sync ring carries xf (stats-critical, first)
            # then lt twice (same weights onto partitions 0-63 for n0 and
            # 64-127 for n1 -> PE row-group pairing) and kk; scalar ring
            # carries the small xs/swall behind its trigger slots.
            xf_t = xfp.tile([128, 4096], BF16)       # x as [(n c), dhw]
            for ch in range(2):
                nc.sync.dma_start(xf_t[:, ch * 2048 : (ch + 1) * 2048],
                                  xf_d.ap()[ch])
            sw_t = consts.tile([128, 1280], BF16)
            nc.sync.dma_start(sw_t[:], sw_d.ap())
            kk_t = consts.tile([128, 3], F32)
            nc.sync.dma_start(kk_t[:], kk_d.ap())
            xs_t = consts.tile([N * C, DL * H * W], BF16)  # [(n c), (dl h w)]
            nc.scalar.dma_start(xs_t[:], xs_d.ap())
            lt_t = consts.tile([128, 4096], BF16)

            # ---- ACT table warm-up: sqrt anchors the set; square/identity
            # are 1-ULP fillers present in the same set. Loads during the x
            # DMA so the engine is table-ready well before inv/copies.
            warm = stats.tile([128, 2], F32)
            nc.vector.memset(warm[:], 1.0)
            nc.scalar.sqrt(warm[:, 0:1], warm[:, 0:1])
            nc.scalar.square(warm[:, 0:1], warm[:, 0:1])
            nc.scalar.activation(warm[:, 0:1], warm[:, 0:1], AF.Identity,
                                 bias=warm[:, 1:2], scale=warm[:, 1:2])
            # il0 weights behind the warmups: the trigger slot lands after
            # the x chunks have claimed the early HBM window
            nc.scalar.dma_start(lt_t[:, 0:2048], lt_d.ap()[0])

            # ---- Stats: per-(n,c) mean/E[x^2] of x. DVE bn_stats (one pass
            # gives mean+var, 512-col max) covers cols 0:3072; ACT covers
            # the last 1024 cols (Identity-sum + Square-sumsq, scales baked
            # to 1/1024 so all P16 cols are "mean-like").
            bn6 = stats.tile([128, 6, 6], F32)
            for s in range(6):
                nc.vector.bn_stats(bn6[:, s], xf_t[:, s * 512 : (s + 1) * 512])
            mv = stats.tile([128, 2], F32)     # [mean_a, var_a] over 0:3072
            nc.vector.bn_aggr(mv[:], bn6[:])
            scr = xfp.tile([128, 1024], BF16)  # ACT throwaway out
            p8 = stats.tile([128, 2], F32)     # [s3, q3] = ch3 sum/sumsq /1024
            nc.scalar.activation(scr[:], xf_t[:, 3072:4096], AF.Identity,
                                 scale=1.0 / 1024.0, accum_out=p8[:, 0:1])
            nc.scalar.activation(scr[:], xf_t[:, 3072:4096], AF.Square,
                                 scale=1.0 / 32.0, accum_out=p8[:, 1:2])
            # combine bn side and ACT side into one "x1024" column each:
            # Sx/1024 = 3*mean_a + s3 ; Sxx/1024 = 3*(var_a+mean_a^2) + q3
            msq = stats.tile([128, 1], F32)
            nc.vector.tensor_mul(msq[:], mv[:, 0:1], mv[:, 0:1])
            ex2 = stats.tile([128, 1], F32)
            nc.vector.tensor_add(ex2[:], mv[:, 1:2], msq[:])
            P16 = stats.tile([128, 3], BF16)   # [Sx/1024, Sxx/1024, 1]
            nc.vector.memset(P16[:, 2:3], 1.0)
            nc.vector.tensor_scalar(P16[:, 0:1], mv[:, 0:1], 3.0, p8[:, 0:1],
                                    op0=ALU.mult, op1=ALU.add)
            nc.vector.tensor_scalar(P16[:, 1:2], ex2[:], 3.0, p8[:, 1:2],
                                    op0=ALU.mult, op1=ALU.add)

            # ---- Fold target (slot 0 of the PSUM ring). The fold matmuls
            # are EMITTED after the first main tile groups so the PE queue
            # doesn't stall on stats before doing any main work.
            # swall: 5 blocks per n ([128,128] each, all columns identical):
            #   b0: 1024*sw/MT*n0, b1: 1024*sww/MT*n0, b2: 1024*2b*sw/MT*n0,
            #   b3: (PV*b)/MT*n0, b4: (PV*b^2+eps*MT/C)/MT*n0; b5..b9 = n1.
            # ps_st cols: 0,1 = mean_tot(n0,n1); 2,3 = E[y^2]+eps (n0,n1).
            ps_st = psp.tile([128, 4], F32, tag="mm")

            def blk(i):
                return sw_t[:, i * 128 : (i + 1) * 128]

            def emit_folds():
                for nq in range(2):
                    o = 5 * nq
                    mc, ec = nq, 2 + nq
                    nc.tensor.matmul(ps_st[:, mc:mc+1], blk(o + 0), P16[:, 0:1],
                                     start=True, stop=False)
                    nc.tensor.matmul(ps_st[:, mc:mc+1], blk(o + 3), P16[:, 2:3],
                                     start=False, stop=True)
                    nc.tensor.matmul(ps_st[:, ec:ec+1], blk(o + 1), P16[:, 1:2],
                                     start=True, stop=False)
                    nc.tensor.matmul(ps_st[:, ec:ec+1], blk(o + 2), P16[:, 0:1],
                                     start=False, stop=False)
                    nc.tensor.matmul(ps_st[:, ec:ec+1], blk(o + 4), P16[:, 2:3],
                                     start=False, stop=True)

            def emit_statmath():
                # var = E[y^2]+eps - mean^2 ; inv = sqrt(1/var)
                # t1 = mean*K2 - K1 (ACT, per-partition kk); c2 = K3 - t1*inv
                msqt = stats.tile([128, 2], F32)
                nc.scalar.square(msqt[:], ps_st[:, 0:2])
                t1 = stats.tile([128, 2], F32)
                nc.scalar.activation(t1[:], ps_st[:, 0:2], AF.Identity,
                                     bias=kk_t[:, 0:1], scale=kk_t[:, 1:2])
                var_t = stats.tile([128, 2], F32)
                nc.vector.tensor_sub(var_t[:], ps_st[:, 2:4], msqt[:])
                rec_t = stats.tile([128, 2], F32)
                nc.vector.reciprocal(rec_t[:], var_t[:])
                inv_t = stats.tile([128, 2], F32)
                nc.scalar.sqrt(inv_t[:], rec_t[:])   # inv = sqrt(1/(var+eps))
                t2 = stats.tile([128, 2], F32)
                nc.vector.tensor_mul(t2[:], t1[:], inv_t[:])
                c2_t = stats.tile([128, 2], F32)
                nc.scalar.activation(c2_t[:], t2[:], AF.Identity,
                                     bias=kk_t[:, 2:3], scale=-1.0)  # K3 - t2
                # il1 weights: trigger after the stat chain so it doesn't
                # block the ACT ops feeding c2; il1 matmuls start much later
                nc.scalar.dma_start(lt_t[:, 2048:4096], lt_d.ap()[1])
                return inv_t, c2_t

            # ---- Main: 128 GEMMs (free=256) + affine copies + out DMA ----
            # lhsT layout: lt[:, pair*128 + 2*o + g] = M0[i=2*g+il, j, k][o, c]
            #   with pair = il*16 + j*4 + k,  psum partition p = 2*o + g.
            # PSUM tile per (n,il,j,dl): cols (k hs w); copy per (j,dl) reads
            # strided (hs, w, k) and writes the ot granule j-slice in 64-elem
            # stride-1 runs. ot granule (n,il,dl) cols: hs*256 + j*64 + w*4+k.
            out_ap = out_d.ap().rearrange(
                "n o (dl g il) ho wo -> n dl il o g (ho wo)", dl=DL, g=2, il=2
            )
            # n-PAIRED matmuls: sample n's weights/rhs live on partitions
            # n*64..n*64+63, so the (n0) and (n1) matmuls of the same
            # (il,j,k,dl) occupy PE row-groups 0 and 64 and run CONCURRENT
            # (tile_position auto-derived from base_partition) -> ~2 cols/cyc.
            # Tiles per (n,il,j,dl) = [128,1024] f32 = 2 banks; 4-slot ring.
            # Copies: scalar owns j0/j1 tiles, vector j2/j3 (one engine per
            # PSUM tile); drain order builds granule (n0,dl0) first.
            PAIRS = [(0, 0), (2, 0), (1, 0), (3, 0),
                     (0, 1), (2, 1), (1, 1), (3, 1)]
            inv_t = c2_t = None

            def emit_mm(ps, n, il, j, dl, k):
                pair = il * 16 + j * 4 + k
                nc.tensor.matmul(
                    ps[:, k * 256 : (k + 1) * 256],
                    lt_t[n * 64 : (n + 1) * 64,
                         pair * 128 : (pair + 1) * 128],
                    xs_t[n * 64 : (n + 1) * 64, dl * 256 : (dl + 1) * 256],
                    start=True, stop=True,
                )

            def emit_mms(ps, n, il, j, dl):
                for k in range(R):
                    emit_mm(ps, n, il, j, dl, k)

            def emit_pair(ps0, ps1, il, j, dl):
                # k-interleaved so the n0 (rows 0-63) and n1 (rows 64-127)
                # matmuls sit adjacent in the PE queue and run concurrently
                for k in range(R):
                    emit_mm(ps0, 0, il, j, dl, k)
                    emit_mm(ps1, 1, il, j, dl, k)

            for il in range(2):
                ots = {}
                for n in range(N):
                    for dl in range(DL):
                        ots[(n, dl)] = otpa.tile(
                            [128, 4096], BF16, tag="ota",
                            name=f"ot_{n}_{il}_{dl}")
                ps_map = {}
                for pi, (j, dl) in enumerate(PAIRS):
                    for n in range(N):
                        ps = psp.tile([128, 1024], F32, tag="mm",
                                      name=f"ps_{il}_{j}_{dl}_{n}")
                        ps_map[(j, dl, n)] = ps
                    if il == 0 and pi == 1:
                        # folds right after the first 4 main matmuls so the
                        # stat chain isn't queued behind the whole prefill;
                        # the tile landing in ps_st's ring slot then has its
                        # WAR-release already in the queue (else deadlock).
                        j0, dl0 = PAIRS[0]
                        emit_mms(ps_map[(j0, dl0, 0)], 0, il, j0, dl0)
                        emit_folds()
                        inv_t, c2_t = emit_statmath()
                        emit_mms(ps_map[(j0, dl0, 1)], 1, il, j0, dl0)
                        emit_pair(ps_map[(j, dl, 0)], ps_map[(j, dl, 1)],
                                  il, j, dl)
                    elif il == 0 and pi == 0:
                        pass  # deferred: emitted right before the folds
                    else:
                        emit_pair(ps_map[(j, dl, 0)], ps_map[(j, dl, 1)],
                                  il, j, dl)
                # copies + granule DMAs; granule (n, dl) needs scalar copies
                # of (j0/j1, dl, n) and vector copies of (j2/j3, dl, n).
                # n-interleaved per j so each tile's WAR-release matches the
                # ring order (no PE queue stalls on later pairs).
                for dl in range(DL):
                    for j in range(R):
                        for n in range(N):
                            src = ps_map[(j, dl, n)][:].rearrange(
                                "p (k hs w) -> p hs w k", k=R, hs=H, w=W
                            )
                            dst = ots[(n, dl)][:].rearrange(
                                "p (hs j w k) -> p hs j w k",
                                hs=H, j=R, w=W, k=R
                            )[:, :, j]
                            # scalar (ACT) copies are ~10% faster than DVE:
                            # give it 9 of the 16 tiles per il-group
                            on_scalar = j < 2 or (j == 3 and dl == 1 and n == 1)
                            if on_scalar:
                                nc.scalar.activation(
                                    dst, src, AF.Identity,
                                    bias=c2_t[:, n : n + 1],
                                    scale=inv_t[:, n : n + 1],
                                )
                            else:
                                nc.vector.tensor_scalar(
                                    dst, src,
                                    inv_t[:, n : n + 1], c2_t[:, n : n + 1],
                                    op0=ALU.mult, op1=ALU.add,
                                )
                    for n in range(N):
                        nc.sync.dma_start(out_ap[n, dl, il], ots[(n, dl)][:])

    nc.compile()
    return nc


def _host_consts(w_ct, b_ct, gamma, beta, w_pw):
    w_ct = np.asarray(w_ct, np.float32).reshape(C, R, R, R)
    b_ct = np.asarray(b_ct, np.float32)
    gamma = np.asarray(gamma, np.float32)
    beta = np.asarray(beta, np.float32)
    w_pw = np.asarray(w_pw, np.float32).reshape(C, C)  # [o, c]

    gw = gamma[:, None, None, None] * w_ct  # [c, i, j, k]
    # lt [c, il, j, k, o, g]; i = 2*g + il; col = pair*128 + 2*o + g
    sc_g0 = gw[:, 0:2]  # g=0: i = il in {0, 1}
    sc_g1 = gw[:, 2:4]  # g=1: i = 2+il
    sc = np.stack([sc_g0, sc_g1], axis=4)  # [c, il, j, k, g]
    lt = (sc[:, :, :, :, None, :]
          * w_pw.T[:, None, None, None, :, None]).reshape(C, 4096)
    # duplicated onto rows 64-127 for the n1 row-group matmuls, then
    # il-chunk-major for the staged loads
    lt = np.concatenate([lt, lt], axis=0)           # [128, 4096]
    lt = np.ascontiguousarray(
        lt.reshape(2 * C, 2, 2048).transpose(1, 0, 2)
    ).astype(ml_dtypes.bfloat16)

    wflat = w_ct.reshape(C, -1)
    sw = 1024.0 * wflat.sum(1) / MT
    sww = 1024.0 * (wflat**2).sum(1) / MT
    tbsw = 1024.0 * 2.0 * b_ct * wflat.sum(1) / MT
    cb = PV * b_ct / MT
    # EPS spread over the C ones so the fold emits E[y^2]+eps directly
    cb2 = (PV * b_ct**2) / MT + EPS / C
    blocks = []
    for nq in range(2):
        for vec in (sw, sww, tbsw, cb, cb2):
            v = np.zeros(128, np.float32)
            v[nq * 64 : (nq + 1) * 64] = vec
            blocks.append(np.repeat(v[:, None], 128, axis=1))
    swall = np.concatenate(blocks, axis=1).astype(ml_dtypes.bfloat16)

    # K1[o]=sum_c wpw*gamma*b, K2[o]=sum_c wpw*gamma, K3[o]=sum_c wpw*beta,
    # expanded to partitions p = 2*o + g. kk col0 = -K1 (ACT bias for
    # t1 = mean*K2 - K1), col1 = K2, col2 = K3.
    k1 = w_pw @ (gamma * b_ct)
    k2 = w_pw @ gamma
    k3 = w_pw @ beta
    k123 = np.repeat(np.stack([-k1, k2, k3], axis=1), 2, axis=0)
    k123 = np.ascontiguousarray(k123, np.float32)
    return lt, swall, k123


def _get_nc():
    if "nc" not in _CACHE:
        _CACHE["nc"] = _build_program()
    return _CACHE["nc"]


def make_in_maps(x, w_ct, b_ct, gamma, beta, w_pw):
    x = np.ascontiguousarray(np.asarray(x, np.float32))
    lt, swall, k123 = _host_consts(w_ct, b_ct, gamma, beta, w_pw)
    x16 = x.astype(ml_dtypes.bfloat16)
    xf = np.ascontiguousarray(
        x16.reshape(N * C, 2, 2048).transpose(1, 0, 2)
    )
    in_maps = []
    for cid in range(NCORES):
        # xs [(n c), (dl h w)]: core's 2 depth planes, sample-major rows so
        # sample n sits on partitions n*64..n*64+63 (PE row-group pairing)
        xs = np.ascontiguousarray(
            x16[:, :, 2 * cid : 2 * cid + 2].reshape(N * C, DL * H * W)
        )
        in_maps.append(dict(xs=xs, xf=xf, lt=lt, swall=swall, k123=k123))
    return in_maps


def assemble(results):
    return np.concatenate(
        [results[cid]["out"] for cid in range(NCORES)], axis=2
    ).astype(np.float32)


def kernel(x, w_ct, b_ct, gamma, beta, w_pw):
    nc = _get_nc()
    in_maps = make_in_maps(x, w_ct, b_ct, gamma, beta, w_pw)
    res = run_bass_kernel_spmd(nc, in_maps, list(range(NCORES))).results
    return assemble(res)
